# revision 12
# baseline (speedup 1.0000x reference)
"""Soft-MoE layer (B=1024, I=512, O=512, E=16) on 8 TRN2 NeuronCores.

Strategy: expert-parallel. Core c owns experts {2c, 2c+1} and computes the
full-batch partial sum
    partial_c[b, o] = sum_{e in core} coeffs[b, e] * (x[b] @ W[e] + bias[e])[o]
followed by an 8-core ReduceScatter(add); core c ends up with output rows
[128c : 128c+128], which the host concatenates.

Per-core compute:
  - x is staged (host-side) transposed as xT [I, B] so the contraction dim I
    lands on SBUF partitions.
  - For each of 8 batch tiles (128 rows) and each owned expert: 4 matmuls
    (K=128 chunks of I) accumulate x@W in PSUM, plus one extra matmul
    (constant 1/128 lhsT  x  host-broadcast bias rhs) that adds bias[e][o] to
    every row of the PSUM tile. This makes the PSUM tile exactly
    (x@W[e] + bias[e]) so coeff-scaling it afterwards is correct.
  - Combine: ScalarE does t = psA * c0 (per-partition scalar), VectorE does
    out = psB * c1 + t (scalar_tensor_tensor). DMA to a DRAM partial buffer.
  - Matmuls run in float32r (full-rate fp32 streaming, 1 cycle/row at N=512).
"""

import numpy as np

import concourse.bass as bass
import concourse.bacc as bacc
import concourse.mybir as mybir
import concourse.tile as tile
from concourse.bass_utils import run_bass_kernel_spmd

B, I, O, E = 1024, 512, 512, 16
NCORES = 8
EPC = E // NCORES  # experts per core
BT = B // 128  # batch tiles
KT = I // 128  # contraction chunks

F32 = mybir.dt.float32
F32R = mybir.dt.float32r

_cache = {}


def _build(niters=1):
    nc = bacc.Bacc(
        "TRN2",
        target_bir_lowering=False,
        debug=False,
        num_devices=NCORES,
    )

    xt_d = nc.dram_tensor("xt", [128, KT, B], F32R, kind="ExternalInput")
    w_d = nc.dram_tensor("w", [128, EPC, KT, O], F32R, kind="ExternalInput")
    bias_d = nc.dram_tensor("biasb", [128, EPC, O], F32R, kind="ExternalInput")
    c2_d = nc.dram_tensor("c2", [128, BT, EPC], F32, kind="ExternalInput")
    ones_d = nc.dram_tensor("ones", [128, 128], F32R, kind="ExternalInput")
    out_d = nc.dram_tensor("out", [128, O], F32, kind="ExternalOutput")

    partial_d = nc.dram_tensor("partial", [B, O], F32)
    rs_d = nc.dram_tensor("rs_out", [128, O], F32)

    with tile.TileContext(nc) as tc:
        with (
            tc.tile_pool(name="const", bufs=1) as const,
            tc.tile_pool(name="psum", bufs=2, space="PSUM") as psum,
            tc.tile_pool(name="stage", bufs=3) as stage,
        ):
            xt_sb = const.tile([128, KT, B], F32R, tag="xt")
            nc.sync.dma_start(xt_sb[:], xt_d[:])
            w_sb = const.tile([128, EPC, KT, O], F32R, tag="w")
            nc.sync.dma_start(w_sb[:], w_d[:])
            bias_sb = const.tile([128, EPC, O], F32R, tag="bias")
            nc.sync.dma_start(bias_sb[:], bias_d[:])
            c2_sb = const.tile([128, BT, EPC], F32, tag="c2")
            nc.sync.dma_start(c2_sb[:], c2_d[:])
            ones_sb = const.tile([128, 128], F32R, tag="ones")
            nc.sync.dma_start(ones_sb[:], ones_d[:])

            for _it in range(niters):
              for i in range(BT):
                bs = slice(128 * i, 128 * (i + 1))
                ps = []
                for e in range(EPC):
                    pse = psum.tile([128, O], F32, tag=f"ps{e}")
                    for k in range(KT):
                        nc.tensor.matmul(
                            pse[:],
                            xt_sb[:, k, bs],
                            w_sb[:, e, k, :],
                            start=(k == 0),
                            stop=False,
                        )
                    # += 1 * bias[e]  (each of the 128 lhsT rows is 1/128)
                    nc.tensor.matmul(
                        pse[:],
                        ones_sb[:],
                        bias_sb[:, e, :],
                        start=False,
                        stop=True,
                    )
                    ps.append(pse)

                t1 = stage.tile([128, O], F32, tag="t1")
                nc.scalar.mul(t1[:], ps[0][:], c2_sb[:, i, 0:1])
                outb = stage.tile([128, O], F32, tag="outb")
                nc.vector.scalar_tensor_tensor(
                    outb[:],
                    ps[1][:],
                    c2_sb[:, i, 1:2],
                    t1[:],
                    mybir.AluOpType.mult,
                    mybir.AluOpType.add,
                )
                nc.sync.dma_start(partial_d[bs, :], outb[:])

              nc.gpsimd.collective_compute(
                  "ReduceScatter",
                  mybir.AluOpType.add,
                  replica_groups=[list(range(NCORES))],
                  ins=[partial_d.ap().opt()],
                  outs=[rs_d.ap().opt()],
              )
              nc.sync.dma_start(out_d[:], rs_d[:])

    nc.compile()
    return nc


def _round_fp32r(a):
    """Round fp32 to fp32r (12 explicit mantissa bits, round-to-nearest) so
    host data matches what the PE datapath consumes."""
    bits = np.ascontiguousarray(a, dtype=np.float32).view(np.uint32)
    r = ((bits.astype(np.uint64) + 0x800) & ~np.uint64(0xFFF)).astype(np.uint32)
    return r.view(np.float32)


def _prep_in_maps(x, coeffs, expert_weights, expert_biases):
    x = _round_fp32r(np.ascontiguousarray(x, dtype=np.float32))
    coeffs = np.ascontiguousarray(coeffs, dtype=np.float32)
    expert_weights = _round_fp32r(
        np.ascontiguousarray(expert_weights, dtype=np.float32)
    )
    expert_biases = _round_fp32r(
        np.ascontiguousarray(expert_biases, dtype=np.float32)
    )

    # xT [I, B] -> [128, KT, B]: partition p, chunk k holds x[:, k*128+p]
    xt = np.ascontiguousarray(x.T.reshape(KT, 128, B).transpose(1, 0, 2))
    ones = np.full((128, 128), 1.0 / 128.0, dtype=np.float32)

    in_maps = []
    for c in range(NCORES):
        es = slice(EPC * c, EPC * (c + 1))
        # W[e][I, O] -> [128, EPC, KT, O]
        w = np.ascontiguousarray(
            expert_weights[es].reshape(EPC, KT, 128, O).transpose(2, 0, 1, 3)
        )
        # bias broadcast to 128 partitions: [128, EPC, O]
        bias = np.ascontiguousarray(
            np.broadcast_to(expert_biases[es][None, :, :], (128, EPC, O))
        )
        # coeffs columns for this core as per-partition scalars: [128, BT, EPC]
        c2 = np.ascontiguousarray(
            coeffs[:, es].reshape(BT, 128, EPC).transpose(1, 0, 2)
        )
        in_maps.append(
            {"xt": xt, "w": w, "biasb": bias, "c2": c2, "ones": ones}
        )
    return in_maps


def _run(inputs, trace=False, **kwargs):
    if "nc" not in _cache:
        _cache["nc"] = _build()
    nc = _cache["nc"]
    in_maps = _prep_in_maps(**inputs)
    res = run_bass_kernel_spmd(
        nc, in_maps, list(range(NCORES)), trace=trace, **kwargs
    )
    out = np.concatenate(
        [np.asarray(res.results[c]["out"]) for c in range(NCORES)], axis=0
    )
    return out.astype(np.float32), res


def kernel(**inputs):
    out, _ = _run(inputs)
    return out


# revision 14
# speedup vs baseline: 3.8100x; 3.8100x over previous
"""Soft-MoE layer (B=1024, I=512, O=512, E=16) on 8 TRN2 NeuronCores.

Strategy: expert-parallel. Core c owns experts {2c, 2c+1} and computes the
full-batch partial sum
    partial_c[b, o] = sum_{e in core} coeffs[b, e] * (x[b] @ W[e] + bias[e])[o]
followed by an 8-core ReduceScatter(add); core c ends up with output rows
[128c : 128c+128], which the host concatenates.

Per-core compute:
  - x is staged (host-side) transposed as xT [I, B] so the contraction dim I
    lands on SBUF partitions.
  - For each of 8 batch tiles (128 rows) and each owned expert: 4 matmuls
    (K=128 chunks of I) accumulate x@W in PSUM, plus one extra matmul
    (constant 1/128 lhsT  x  host-broadcast bias rhs) that adds bias[e][o] to
    every row of the PSUM tile. This makes the PSUM tile exactly
    (x@W[e] + bias[e]) so coeff-scaling it afterwards is correct.
  - Combine: ScalarE does t = psA * c0 (per-partition scalar), VectorE does
    out = psB * c1 + t (scalar_tensor_tensor). DMA to a DRAM partial buffer.
  - Matmuls run in float32r (full-rate fp32 streaming, 1 cycle/row at N=512).
"""

import numpy as np

import concourse.bass as bass
import concourse.bacc as bacc
import concourse.mybir as mybir
import concourse.tile as tile
from concourse.bass_utils import run_bass_kernel_spmd

B, I, O, E = 1024, 512, 512, 16
NCORES = 8
EPC = E // NCORES  # experts per core
BT = B // 128  # batch tiles
KT = I // 128  # contraction chunks

F32 = mybir.dt.float32
F32R = mybir.dt.float32r

_cache = {}


def _build(niters=1, with_rs=True):
    nc = bacc.Bacc(
        "TRN2",
        target_bir_lowering=False,
        debug=False,
        num_devices=NCORES,
    )

    xt_d = nc.dram_tensor("xt", [128, KT, B], F32R, kind="ExternalInput")
    w_d = nc.dram_tensor("w", [128, EPC, KT, O], F32R, kind="ExternalInput")
    bias_d = nc.dram_tensor("biasb", [128, EPC, O], F32R, kind="ExternalInput")
    c2_d = nc.dram_tensor("c2", [128, BT, EPC], F32, kind="ExternalInput")
    ones_d = nc.dram_tensor("ones", [128, 128], F32R, kind="ExternalInput")
    out_d = nc.dram_tensor("out", [128, O], F32, kind="ExternalOutput")

    partial_d = nc.dram_tensor("partial", [B, O], F32)
    rs_d = nc.dram_tensor("rs_out", [128, O], F32)

    with tile.TileContext(nc) as tc:
        with (
            tc.tile_pool(name="const", bufs=1) as const,
            tc.tile_pool(name="psum", bufs=2, space="PSUM") as psum,
            tc.tile_pool(name="stage", bufs=3) as stage,
        ):
            xt_sb = const.tile([128, KT, B], F32R, tag="xt")
            nc.sync.dma_start(xt_sb[:], xt_d[:])
            w_sb = const.tile([128, EPC, KT, O], F32R, tag="w")
            nc.sync.dma_start(w_sb[:], w_d[:])
            bias_sb = const.tile([128, EPC, O], F32R, tag="bias")
            nc.sync.dma_start(bias_sb[:], bias_d[:])
            c2_sb = const.tile([128, BT, EPC], F32, tag="c2")
            nc.sync.dma_start(c2_sb[:], c2_d[:])
            ones_sb = const.tile([128, 128], F32R, tag="ones")
            nc.sync.dma_start(ones_sb[:], ones_d[:])

            for _it in range(niters):
              for i in range(BT):
                bs = slice(128 * i, 128 * (i + 1))
                ps = []
                for e in range(EPC):
                    pse = psum.tile([128, O], F32, tag=f"ps{e}")
                    for k in range(KT):
                        nc.tensor.matmul(
                            pse[:],
                            xt_sb[:, k, bs],
                            w_sb[:, e, k, :],
                            start=(k == 0),
                            stop=False,
                        )
                    # += 1 * bias[e]  (each of the 128 lhsT rows is 1/128)
                    nc.tensor.matmul(
                        pse[:],
                        ones_sb[:],
                        bias_sb[:, e, :],
                        start=False,
                        stop=True,
                    )
                    ps.append(pse)

                t1 = stage.tile([128, O], F32, tag="t1")
                nc.scalar.mul(t1[:], ps[0][:], c2_sb[:, i, 0:1])
                outb = stage.tile([128, O], F32, tag="outb")
                nc.vector.scalar_tensor_tensor(
                    outb[:],
                    ps[1][:],
                    c2_sb[:, i, 1:2],
                    t1[:],
                    mybir.AluOpType.mult,
                    mybir.AluOpType.add,
                )
                nc.sync.dma_start(partial_d[bs, :], outb[:])

              if with_rs:
                  nc.gpsimd.collective_compute(
                      "ReduceScatter",
                      mybir.AluOpType.add,
                      replica_groups=[list(range(NCORES))],
                      ins=[partial_d.ap().opt()],
                      outs=[rs_d.ap().opt()],
                  )
                  nc.sync.dma_start(out_d[:], rs_d[:])
              else:
                  # timing-only variant: keep an output dependency, no RS
                  nc.sync.dma_start(out_d[:], partial_d[0:128, :])

    nc.compile()
    return nc


def _round_fp32r(a):
    """Round fp32 to fp32r (12 explicit mantissa bits, round-to-nearest) so
    host data matches what the PE datapath consumes."""
    bits = np.ascontiguousarray(a, dtype=np.float32).view(np.uint32)
    r = ((bits.astype(np.uint64) + 0x800) & ~np.uint64(0xFFF)).astype(np.uint32)
    return r.view(np.float32)


def _prep_in_maps(x, coeffs, expert_weights, expert_biases):
    x = _round_fp32r(np.ascontiguousarray(x, dtype=np.float32))
    coeffs = np.ascontiguousarray(coeffs, dtype=np.float32)
    expert_weights = _round_fp32r(
        np.ascontiguousarray(expert_weights, dtype=np.float32)
    )
    expert_biases = _round_fp32r(
        np.ascontiguousarray(expert_biases, dtype=np.float32)
    )

    # xT [I, B] -> [128, KT, B]: partition p, chunk k holds x[:, k*128+p]
    xt = np.ascontiguousarray(x.T.reshape(KT, 128, B).transpose(1, 0, 2))
    ones = np.full((128, 128), 1.0 / 128.0, dtype=np.float32)

    in_maps = []
    for c in range(NCORES):
        es = slice(EPC * c, EPC * (c + 1))
        # W[e][I, O] -> [128, EPC, KT, O]
        w = np.ascontiguousarray(
            expert_weights[es].reshape(EPC, KT, 128, O).transpose(2, 0, 1, 3)
        )
        # bias broadcast to 128 partitions: [128, EPC, O]
        bias = np.ascontiguousarray(
            np.broadcast_to(expert_biases[es][None, :, :], (128, EPC, O))
        )
        # coeffs columns for this core as per-partition scalars: [128, BT, EPC]
        c2 = np.ascontiguousarray(
            coeffs[:, es].reshape(BT, 128, EPC).transpose(1, 0, 2)
        )
        in_maps.append(
            {"xt": xt, "w": w, "biasb": bias, "c2": c2, "ones": ones}
        )
    return in_maps


def _run(inputs, trace=False, **kwargs):
    if "nc" not in _cache:
        _cache["nc"] = _build()
    nc = _cache["nc"]
    in_maps = _prep_in_maps(**inputs)
    res = run_bass_kernel_spmd(
        nc, in_maps, list(range(NCORES)), trace=trace, **kwargs
    )
    out = np.concatenate(
        [np.asarray(res.results[c]["out"]) for c in range(NCORES)], axis=0
    )
    return out.astype(np.float32), res


def kernel(**inputs):
    out, _ = _run(inputs)
    return out


# revision 17
# speedup vs baseline: 12.9377x; 3.3957x over previous
"""Soft-MoE layer (B=1024, I=512, O=512, E=16) on 8 TRN2 NeuronCores.

Strategy: output-column sharding (no collectives). Core c owns output
columns [64c : 64c+64] and computes, for the full batch and ALL 16 experts,
    out[b, oc] = sum_e coeffs[b, e] * (x[b] @ W[e][:, oc] + bias[e][oc])
The host concatenates the 8 column slices. ncfw collectives measured
~100+us for a 2MB 8-rank ReduceScatter (latency-floor dominated), so the
expert reduction is done locally on DVE instead: PE computes per-expert
partials for the core's 64 columns, packed 8-experts-per-matmul along the
free dim (N=512, full PE efficiency), then DVE applies the per-sample
coefficients (stride-0 broadcast APs) and reduces over experts.

Details:
  - x is staged host-side transposed (xT) so the contraction dim I lands on
    SBUF partitions; matmuls run in float32r (full-rate fp32 streaming, 12
    mantissa bits — host pre-rounds operands to match).
  - bias[e] is folded into the PSUM accumulation with one extra matmul per
    psum tile: lhsT = const 1/128, rhs = host-broadcast biases. The PSUM
    tile then holds exactly (x@W[e] + bias[e]) per expert block, so the
    coefficient weighting afterwards is correct.
"""

import numpy as np

import concourse.bass as bass
import concourse.bacc as bacc
import concourse.mybir as mybir
import concourse.tile as tile
from concourse.bass_utils import run_bass_kernel_spmd

B, I, O, E = 1024, 512, 512, 16
NCORES = 8
OC = O // NCORES  # output columns per core = 64
BT = B // 128  # batch tiles = 8
KT = I // 128  # contraction chunks = 4
EH = E // 2  # experts per psum half = 8

F32 = mybir.dt.float32
F32R = mybir.dt.float32r

_cache = {}


def _build(loop_n=None):
    """loop_n: if set, wrap the per-iteration body in a hardware For_i loop
    (benchmark amplification only)."""
    nc = bacc.Bacc(
        "TRN2",
        target_bir_lowering=False,
        debug=False,
        num_devices=NCORES,
    )

    xt_d = nc.dram_tensor("xt", [128, KT, B], F32R, kind="ExternalInput")
    w_d = nc.dram_tensor("w", [128, KT, E, OC], F32R, kind="ExternalInput")
    bias_d = nc.dram_tensor("biasb", [128, E, OC], F32R, kind="ExternalInput")
    c2_d = nc.dram_tensor("c2", [128, BT, E], F32, kind="ExternalInput")
    ones_d = nc.dram_tensor("ones", [128, 128], F32R, kind="ExternalInput")
    out_d = nc.dram_tensor("out", [B, OC], F32, kind="ExternalOutput")

    with tile.TileContext(nc) as tc:
        with (
            tc.tile_pool(name="const", bufs=1) as const,
            tc.tile_pool(name="psum", bufs=2, space="PSUM") as psum,
            tc.tile_pool(name="stage", bufs=3) as stage,
        ):
            xt_sb = const.tile([128, KT, B], F32R, tag="xt")
            nc.sync.dma_start(xt_sb[:], xt_d[:])
            w_sb = const.tile([128, KT, E, OC], F32R, tag="w")
            nc.sync.dma_start(w_sb[:], w_d[:])
            bias_sb = const.tile([128, E, OC], F32R, tag="bias")
            nc.sync.dma_start(bias_sb[:], bias_d[:])
            c2_sb = const.tile([128, BT, E], F32, tag="c2")
            nc.sync.dma_start(c2_sb[:], c2_d[:])
            ones_sb = const.tile([128, 128], F32R, tag="ones")
            nc.sync.dma_start(ones_sb[:], ones_d[:])

            def body():
                for i in range(BT):
                    bs = slice(128 * i, 128 * (i + 1))
                    m = stage.tile([128, E, OC], F32, tag="m")
                    for h in range(2):
                        es = slice(EH * h, EH * (h + 1))
                        pse = psum.tile([128, EH, OC], F32, tag=f"ps{h}")
                        for k in range(KT):
                            nc.tensor.matmul(
                                pse[:],
                                xt_sb[:, k, bs],
                                w_sb[:, k, es, :],
                                start=(k == 0),
                                stop=False,
                            )
                        # += 1 * bias[e] (each of the 128 lhsT rows is 1/128)
                        nc.tensor.matmul(
                            pse[:],
                            ones_sb[:],
                            bias_sb[:, es, :],
                            start=False,
                            stop=True,
                        )
                        # m[:, e, :] = pse[:, e, :] * coeffs[b, e]
                        cb = (
                            c2_sb[:, i, es]
                            .unsqueeze(2)
                            .broadcast_to([128, EH, OC])
                        )
                        nc.vector.tensor_mul(m[:, es, :], pse[:], cb)
                    outb = stage.tile([128, OC], F32, tag="outb")
                    nc.vector.tensor_reduce(
                        outb[:],
                        m[:].transpose([0, 2, 1]),
                        mybir.AxisListType.X,
                        mybir.AluOpType.add,
                    )
                    nc.sync.dma_start(out_d[bs, :], outb[:])

            if loop_n is not None:
                with tc.For_i(0, loop_n, 1):
                    body()
            else:
                body()

    nc.compile()
    return nc


def _round_fp32r(a):
    """Round fp32 to fp32r (12 explicit mantissa bits, round-to-nearest) so
    host data matches what the PE datapath consumes."""
    bits = np.ascontiguousarray(a, dtype=np.float32).view(np.uint32)
    r = ((bits.astype(np.uint64) + 0x800) & ~np.uint64(0xFFF)).astype(np.uint32)
    return r.view(np.float32)


def _prep_in_maps(x, coeffs, expert_weights, expert_biases):
    x = _round_fp32r(np.ascontiguousarray(x, dtype=np.float32))
    coeffs = np.ascontiguousarray(coeffs, dtype=np.float32)
    expert_weights = _round_fp32r(
        np.ascontiguousarray(expert_weights, dtype=np.float32)
    )
    expert_biases = _round_fp32r(
        np.ascontiguousarray(expert_biases, dtype=np.float32)
    )

    # xT [I, B] -> [128, KT, B]: partition p, chunk k holds x[:, k*128+p]
    xt = np.ascontiguousarray(x.T.reshape(KT, 128, B).transpose(1, 0, 2))
    ones = np.full((128, 128), 1.0 / 128.0, dtype=np.float32)
    # coeffs as per-partition scalars: [128, BT, E]
    c2 = np.ascontiguousarray(coeffs.reshape(BT, 128, E).transpose(1, 0, 2))

    in_maps = []
    for c in range(NCORES):
        ocs = slice(OC * c, OC * (c + 1))
        # W[e][I, oc] -> [128, KT, E, OC]
        w = np.ascontiguousarray(
            expert_weights[:, :, ocs]
            .reshape(E, KT, 128, OC)
            .transpose(2, 1, 0, 3)
        )
        # bias broadcast down partitions: [128, E, OC]
        biasb = np.ascontiguousarray(
            np.broadcast_to(expert_biases[None, :, ocs], (128, E, OC))
        )
        in_maps.append(
            {"xt": xt, "w": w, "biasb": biasb, "c2": c2, "ones": ones}
        )
    return in_maps


def _run(inputs, **kwargs):
    if "nc" not in _cache:
        _cache["nc"] = _build()
    nc = _cache["nc"]
    in_maps = _prep_in_maps(**inputs)
    res = run_bass_kernel_spmd(nc, in_maps, list(range(NCORES)), **kwargs)
    out = np.concatenate(
        [np.asarray(res.results[c]["out"]) for c in range(NCORES)], axis=1
    )
    return out.astype(np.float32), res


def kernel(**inputs):
    out, _ = _run(inputs)
    return out
